# revision 23
# baseline (speedup 1.0000x reference)
"""Trainium2 Bass kernel for the quantized BasicBlock (nn_BasicBlock_15436112462307).

Strategy
--------
Data-parallel over batch: 64 images -> 8 cores x 8 images. Weights/BN replicated.

fake_quant makes every conv operand an exact small integer (-7..7) times a
global fp32 scale.  We factor the scales out on the host and feed pure
integers to the PE as fp8e4 (integers <=7 are exact in fp8e4), using
perf_mode=DoubleRow so one matmul contracts all 256 input channels
(lhsT [128,2,128] / rhs [128,2,N]) at 2x fp8 rate.  PSUM accumulates the
integer dot products exactly in fp32, so the conv itself is EXACT; all
rounding happens only in the per-channel epilogues, which replicate the
reference's fp32 arithmetic.

Spatial layout: each 28x28 image is zero-padded to 30x30 and flattened, so
every 3x3 conv tap is a pure diagonal shift in the flat index -> conv =
9 accumulating matmuls over contiguous windows.  We compute 30-wide output
rows (2 garbage columns per row) and discard the garbage in the epilogue APs.

Epilogue 1 (conv1 -> conv2 input):  q2 = rne(clip(P1*(7*sx*sw1*inv1) + 7*b1, +-7))
using the fp32 magic-number trick (+-1.5*2^23) for round-to-nearest-even;
the result is an exact integer written directly as fp8 into the padded conv2
input buffer.  The activation fake-quant scale alpha2 = max|hardtanh(...)| is
1.0 whenever anything clips (always, for this distribution); the kernel
computes max|.| on device and the host verifies it is exactly 7.0, falling
back to an exact numpy implementation otherwise.

Epilogue 2: y = clip(P2*(s2*sw2*inv2) + (x*inv2 + b2), +-1); the residual
affine x*inv2+b2 is precomputed on the host, so the device does one fused
scalar_tensor_tensor (scale+add) plus the clip on VectorE.

Inputs stream in four chained DMA stages ordered by first use (conv1/cot0
weights + images 0-1 first) so the PE starts ~11us into the kernel; the
measured span is ~125us, with the tensor engine at its DoubleRow streaming
floor (~105us, zero stalls) and fixed preamble/exit-barrier around it.
"""

import numpy as np
import ml_dtypes

EPS = np.float32(1e-5)
NCORES = 8
B, C, H, W = 64, 256, 28, 28
BC = B // NCORES            # images per core
IMS = 912                   # padded (30x30=900) image stride, multiple of 16
NT = 420                    # matmul N: 14 padded rows x 30
MAGIC = np.float32(12582912.0)  # 1.5 * 2^23
F8NP = ml_dtypes.float8_e4m3

WB = 36 * 2 * 128                   # 9216 bytes/partition of int weights
WG = 2 * 128                        # one weight group (tap)
VB = 48                             # 40B of fp32 epilogue vecs + 8B pad
IMB = 2 * IMS                       # one image (both channel halves)
X1B = BC * IMB                      # 14592 bytes/partition of int inputs
# staged layout: [w(conv1,cot0) | vec | imgs0-1 | w rest | imgs2-7]
W0B = 9 * WG
VOFF = W0B
X0OFF = W0B + VB
WROFF = X0OFF + 2 * IMB
X2OFF = WROFF + (WB - W0B)
INPB = X2OFF + 6 * IMB

_BUILT = None  # cached (nc,) so repeat calls skip IR building + compile


# ----------------------------------------------------------------- host math
def _quant_int(v):
    """Exact replica of the reference fake_quant grid; returns integer part."""
    alpha = np.float32(np.float32(np.max(np.abs(v))) + np.float32(1e-12))
    scale = np.float32(alpha / np.float32(7.0))
    q = np.round(np.clip(v, -alpha, alpha) / scale).astype(np.float32)
    return q, scale


def _fold_bn(gamma, beta, mean, var):
    gamma = np.asarray(gamma, np.float32)
    beta = np.asarray(beta, np.float32)
    mean = np.asarray(mean, np.float32)
    var = np.asarray(var, np.float32)
    inv = (gamma / np.sqrt(var + EPS)).astype(np.float32)
    b = (beta - mean * inv).astype(np.float32)
    return inv, b


# ------------------------------------------------------------------ bass IR
def _build():
    global _BUILT
    if _BUILT is not None:
        return _BUILT
    import concourse.bacc as bacc
    import concourse.tile as tile
    from concourse import mybir
    from contextlib import ExitStack

    f32 = mybir.dt.float32
    f8 = mybir.dt.float8e4
    AF = mybir.ActivationFunctionType
    OP = mybir.AluOpType
    DR = mybir.MatmulPerfMode.DoubleRow
    AX = mybir.AxisListType

    SA1, SA, SB1 = X0OFF + IMB, WROFF, X2OFF + 2 * IMB
    nc = bacc.Bacc("TRN2", target_bir_lowering=False, debug=False)
    inpa_d = nc.dram_tensor("inpa", [128, SA1], f8, kind="ExternalInput").ap()
    inpa2_d = nc.dram_tensor("inpa2", [128, SA - SA1], f8, kind="ExternalInput").ap()
    inpb1_d = nc.dram_tensor("inpb1", [128, SB1 - SA], f8, kind="ExternalInput").ap()
    inpb2_d = nc.dram_tensor("inpb2", [128, INPB - SB1], f8, kind="ExternalInput").ap()
    r_d = nc.dram_tensor("resid", [128, 2, BC, 2, 14, 28], f32, kind="ExternalInput").ap()
    y_d = nc.dram_tensor("y", [2, 128, BC, 2, 14, 28], f32, kind="ExternalOutput").ap()
    am_d = nc.dram_tensor("amax", [128, 4], f32, kind="ExternalOutput").ap()

    with tile.TileContext(nc) as tc, ExitStack() as ctx:
        const = ctx.enter_context(tc.tile_pool(name="const", bufs=1))
        psum = ctx.enter_context(tc.tile_pool(name="psum", bufs=8, space="PSUM"))
        ep1 = ctx.enter_context(tc.tile_pool(name="ep1", bufs=4))
        ep2 = ctx.enter_context(tc.tile_pool(name="ep2", bufs=4))
        yp = ctx.enter_context(tc.tile_pool(name="yp", bufs=3))

        inp_sb = const.tile([128, INPB], f8, tag="inp")
        x2_sb = const.tile([128, BC, 2, IMS], f8, tag="x2")
        rs_sb = const.tile([128, 2, BC, 2, 14, 28], f32, tag="rs")
        am_sb = const.tile([128, 4], f32, tag="am")

        vecv = inp_sb[:, VOFF:VOFF + 40].bitcast(f32)      # [128, 10] f32

        def w_ap(g):   # [128, 2, 128] tap g in (ci,cot)-major order
            off = g * WG if g < 9 else WROFF + (g - 9) * WG
            return inp_sb[:, off:off + WG].rearrange("p (r m) -> p r m", r=2)

        def x1_ap(b):  # [128, 2, IMS] image b
            off = X0OFF + b * IMB if b < 2 else X2OFF + (b - 2) * IMB
            return inp_sb[:, off:off + IMB].rearrange("p (r s) -> p r s", r=2)

        # HAM pre-warm: run junk matmuls on zeroed SBUF during the input-DMA
        # window so the PE clock gate is already at 2.4GHz (warm) when the
        # first real matmul issues (~3.4us of sustained activity required).
        wj = const.tile([128, 256], f8, tag="wj")
        nc.vector.memset(wj[:], 0.0)
        jl = wj[:].rearrange("p (r m) -> p r m", r=2)
        jp = psum.tile([128, NT], f32, tag="pt", name="jp")
        for _ in range(28):
            nc.tensor.matmul(jp[:, 0:128], jl, jl, start=True, stop=True,
                             perf_mode=DR)

        from concourse.tile_rust import add_dep_helper
        dma_a = nc.sync.dma_start(inp_sb[:, 0:SA1], inpa_d)
        dma_a2 = nc.sync.dma_start(inp_sb[:, SA1:SA], inpa2_d)
        dma_b1 = nc.sync.dma_start(inp_sb[:, SA:SB1], inpb1_d)
        dma_b2 = nc.sync.dma_start(inp_sb[:, SB1:INPB], inpb2_d)
        dma_r = nc.sync.dma_start(rs_sb[:], r_d)
        for a, b in ((dma_b1, dma_a2), (dma_b2, dma_b1), (dma_r, dma_b2)):
            add_dep_helper(a.ins, b.ins, sync=True,
                           reason="stage input DMAs by first-use order")
        nc.gpsimd.memset(x2_sb[:], 0.0)

        def vcol(i):
            return vecv[:, i : i + 1]

        def valid(ap420):  # [128,420] -> [128,14,28] dropping 2 garbage cols/row
            return ap420.rearrange("p (h w) -> p h w", w=30)[:, :, :28]

        for ci, src in ((0, None), (1, x2_sb)):
            # conv2 tapers to 1-image phases so the final epilogue tail is short
            groups = ([(b0, 2) for b0 in range(0, BC, 2)] if ci == 0 else
                      [(0, 2), (2, 2), (4, 2), (6, 1), (7, 1)])
            for b0, gsz in groups:
                for cot in range(2):
                    pts = {}
                    # -- 9 taps x (gsz images x 2 row-halves) per weight --
                    for k in range(9):
                        off = (k // 3) * 30 + (k % 3)
                        lhsT = w_ap((ci * 2 + cot) * 9 + k)
                        for bb in range(gsz):
                            b = b0 + bb
                            for hb in range(2):
                                if k == 0:
                                    pts[(bb, hb)] = psum.tile(
                                        [128, NT], f32, tag="pt", name="pt")
                                s = hb * NT + off
                                rhs = (x1_ap(b) if ci == 0 else
                                       src[:, b, :, :])[:, :, s : s + NT]
                                nc.tensor.matmul(
                                    pts[(bb, hb)][:], lhsT, rhs,
                                    start=(k == 0), stop=(k == 8), perf_mode=DR)
                    # ---- epilogues for this phase's psum tiles ----
                    for bb in range(gsz):
                        b = b0 + bb
                        if ci == 1:
                            yb = yp.tile([128, 2, 14, 28], f32, tag="yb", name="yb")
                        for hb in range(2):
                            pt3 = valid(pts[(bb, hb)][:])
                            if ci == 0:
                                # t=P*a1+b1p ; clip +-7 ; +-MAGIC rne -> fp8
                                t1 = ep1.tile([128, 14, 28], f32, tag="t1", name="t1")
                                nc.scalar.activation(
                                    t1[:], pt3, AF.Identity,
                                    bias=vcol(2 + cot), scale=vcol(0 + cot))
                                t2 = ep1.tile([128, 14, 28], f32, tag="t2", name="t2")
                                nc.vector.tensor_scalar(
                                    t2[:], t1[:], 7.0, -7.0, op0=OP.min, op1=OP.max)
                                if bb == 0 and hb == 0 and b0 in (0, 2):
                                    # any tile hitting exactly 7.0 proves
                                    # alpha2 == 1.0 globally (clip bound)
                                    idx = (b0 // 2) * 2 + cot
                                    nc.vector.tensor_reduce(
                                        am_sb[:, idx : idx + 1], t2[:], op=OP.max,
                                        axis=AX.XY, apply_absolute_value=True)
                                t3 = ep1.tile([128, 14, 28], f32, tag="t3", name="t3")
                                nc.scalar.activation(
                                    t3[:], t2[:], AF.Copy, bias=float(MAGIC), scale=1.0)
                                dst = valid(
                                    x2_sb[:, b, cot, hb * NT + 31 : hb * NT + 31 + NT])
                                nc.vector.tensor_scalar(
                                    dst, t3[:], -float(MAGIC), None, op0=OP.add)
                            else:
                                # y = clip(P2*c2 + (x*inv2 + b2), +-1);
                                # the residual affine is precomputed on host
                                u3 = ep2.tile([128, 14, 28], f32, tag="u3", name="u3")
                                nc.vector.scalar_tensor_tensor(
                                    u3[:], pt3, vcol(4 + cot), rs_sb[:, cot, b, hb],
                                    op0=OP.mult, op1=OP.add)
                                nc.vector.tensor_scalar(
                                    yb[:, hb], u3[:], 1.0, -1.0,
                                    op0=OP.min, op1=OP.max)
                                if gsz == 1:
                                    nc.sync.dma_start(y_d[cot, :, b, hb],
                                                      yb[:, hb])
                        if ci == 1 and gsz > 1:
                            nc.sync.dma_start(y_d[cot, :, b], yb[:])
            if ci == 0:
                nc.sync.dma_start(am_d, am_sb[:])

    nc.compile()
    _dedupe_ldweights(nc)
    _BUILT = (nc,)
    return _BUILT


# ------------------------------------------------------------- input packing
def _prep(x, w1, w2, inv1, b1, inv2, b2):
    xi, s_x = _quant_int(x)
    w1i, s_w1 = _quant_int(w1)
    w2i, s_w2 = _quant_int(w2)

    xi8 = xi.astype(F8NP)
    tmp = np.zeros((NCORES, BC, 2, 128, 30, 30), F8NP)
    tmp[:, :, :, :, 1:29, 1:29] = xi8.reshape(NCORES, BC, 2, 128, 28, 28)
    x1_all = np.zeros((NCORES, 128, BC, 2, IMS), F8NP)
    x1_all[..., :900] = tmp.transpose(0, 3, 1, 2, 4, 5).reshape(
        NCORES, 128, BC, 2, 900)

    def wpack(wi):
        # w[cot*128+m, r*128+p, kh, kw] -> [p, (cot,k), r, m]
        v = wi.reshape(2, 128, 2, 128, 9)          # cot, m, r, p, k
        v = v.transpose(3, 0, 4, 2, 1)             # p, cot, k, r, m
        return v.reshape(128, 18, 2, 128).astype(F8NP)

    w_all = np.concatenate([wpack(w1i), wpack(w2i)], axis=1).reshape(128, WB)

    s2 = np.float32(np.float32(1.0) / np.float32(7.0))
    a1 = (np.float32(7.0) * s_x * s_w1 * inv1).astype(np.float32)
    b1p = (np.float32(7.0) * b1).astype(np.float32)
    c2 = (s2 * s_w2 * inv2).astype(np.float32)
    cols = [a1[:128], a1[128:], b1p[:128], b1p[128:], c2[:128], c2[128:],
            inv2[:128], inv2[128:], b2[:128], b2[128:]]
    vec8 = np.zeros((128, VB), F8NP)
    vec8[:, :40] = np.ascontiguousarray(
        np.stack(cols, axis=1).astype(np.float32)).view(F8NP)

    # residual affine x*inv2 + b2, precomputed -> [cores, 128(m), 2(cot), BC, ...]
    rs2 = (x * inv2[None, :, None, None] + b2[None, :, None, None]).astype(np.float32)
    resid = rs2.reshape(NCORES, BC, 2, 128, 2, 14, 28).transpose(0, 3, 2, 1, 4, 5, 6)
    resid = np.ascontiguousarray(resid)

    in_maps = []
    for i in range(NCORES):
        x1i = x1_all[i].reshape(128, X1B)
        inpa = np.concatenate([w_all[:, :W0B], vec8, x1i[:, :IMB]], axis=1)
        inpa2 = x1i[:, IMB:2 * IMB]
        inpb1 = np.concatenate([w_all[:, W0B:], x1i[:, 2 * IMB:4 * IMB]], axis=1)
        inpb2 = x1i[:, 4 * IMB:]
        in_maps.append({"inpa": np.ascontiguousarray(inpa),
                        "inpa2": np.ascontiguousarray(inpa2),
                        "inpb1": np.ascontiguousarray(inpb1),
                        "inpb2": np.ascontiguousarray(inpb2),
                        "resid": resid[i]})
    return in_maps, (xi, w1i, w2i, s_x, s_w1, s_w2, s2)


# ------------------------------------------------------- exact numpy fallback
def _conv3x3_int(xint, wint):
    Bn, Cn, Hn, Wn = xint.shape
    xp = np.zeros((Bn, Cn, Hn + 2, Wn + 2), np.float64)
    xp[:, :, 1:-1, 1:-1] = xint
    out = np.zeros((Bn, wint.shape[0], Hn, Wn), np.float64)
    w64 = wint.astype(np.float64)
    for kh in range(3):
        for kw in range(3):
            out += np.einsum("bchw,oc->bohw", xp[:, :, kh:kh + Hn, kw:kw + Wn],
                             w64[:, :, kh, kw], optimize=True)
    return out.astype(np.float32)


def _numpy_path(x, q, inv1, b1, inv2, b2):
    """Exact replica handling arbitrary alpha2 (never expected to run)."""
    xi, w1i, w2i, s_x, s_w1, s_w2, _ = q
    P1 = _conv3x3_int(xi, w1i)
    h = (P1 * (s_x * s_w1 * inv1)[None, :, None, None]).astype(np.float32)
    h = (h + b1[None, :, None, None]).astype(np.float32)
    h = np.clip(h, np.float32(-1.0), np.float32(1.0))
    alpha2 = np.float32(np.abs(h).max())
    s2 = np.float32(alpha2 / np.float32(7.0))
    x2 = np.round(np.clip(h, -alpha2, alpha2) / s2).astype(np.float32)
    P2 = _conv3x3_int(x2, w2i)
    u = (P2 * (s2 * s_w2 * inv2)[None, :, None, None]).astype(np.float32)
    u = (u + (x * inv2[None, :, None, None] + b2[None, :, None, None])).astype(np.float32)
    return np.clip(u, np.float32(-1.0), np.float32(1.0))


# ------------------------------------------------------------------- kernel
def _dedupe_ldweights(nc):
    """Drop InstLdweights that reload the stationary operand already in the
    PE array (consecutive matmuls here reuse one weight 8x).  Safe because
    Ldweights carry no semaphore updates; ones carrying waits are kept."""
    for f in nc.m.functions:
        for blk in f.blocks:
            il = blk.instructions
            keep, last_sig, removed = [], None, 0
            for ins in il:
                tn = type(ins).__name__
                if tn == "InstLdweights":
                    sig = (str(ins.ins), str(ins.perf_mode),
                           str(ins.tile_position), str(ins.is_transpose))
                    plain = ("wait:" not in str(ins)
                             and "update:" not in str(ins))
                    if sig == last_sig and plain:
                        removed += 1
                        continue
                    last_sig = sig
                elif tn in ("InstMatmult", "InstEventSemaphore", "InstDrain"):
                    pass                     # none of these clobber loaded weights
                elif str(getattr(ins, "engine", "")).endswith("PE"):
                    last_sig = None          # conservative reset on other PE ops
                keep.append(ins)
            if removed:
                il[:] = keep


def _run(in_maps, trace=False, tmpdir=None):
    from concourse.bass_utils import run_bass_kernel_spmd
    (nc,) = _build()
    return run_bass_kernel_spmd(nc, in_maps, list(range(NCORES)), trace=trace,
                                tmpdir=tmpdir)


def kernel(x, w1, bn1_gamma, bn1_beta, bn1_mean, bn1_var,
           w2, bn2_gamma, bn2_beta, bn2_mean, bn2_var):
    x = np.asarray(x, np.float32)
    w1 = np.asarray(w1, np.float32)
    w2 = np.asarray(w2, np.float32)
    inv1, b1 = _fold_bn(bn1_gamma, bn1_beta, bn1_mean, bn1_var)
    inv2, b2 = _fold_bn(bn2_gamma, bn2_beta, bn2_mean, bn2_var)

    in_maps, q = _prep(x, w1, w2, inv1, b1, inv2, b2)
    res = _run(in_maps)

    amax = np.max([r["amax"] for r in res.results])
    if not np.float32(amax) == np.float32(7.0):
        return _numpy_path(x, q, inv1, b1, inv2, b2)

    ys = np.stack([r["y"] for r in res.results])      # [cores, 2, 128, BC, 2,14,28]
    ys = ys.reshape(NCORES, 2, 128, BC, 784)
    return ys.transpose(0, 3, 1, 2, 4).reshape(B, C, H, W).copy()


# revision 24
# speedup vs baseline: 1.0049x; 1.0049x over previous
"""Trainium2 Bass kernel for the quantized BasicBlock (nn_BasicBlock_15436112462307).

Strategy
--------
Data-parallel over batch: 64 images -> 8 cores x 8 images. Weights/BN replicated.

fake_quant makes every conv operand an exact small integer (-7..7) times a
global fp32 scale.  We factor the scales out on the host and feed pure
integers to the PE as fp8e4 (integers <=7 are exact in fp8e4), using
perf_mode=DoubleRow so one matmul contracts all 256 input channels
(lhsT [128,2,128] / rhs [128,2,N]) at 2x fp8 rate.  PSUM accumulates the
integer dot products exactly in fp32, so the conv itself is EXACT; all
rounding happens only in the per-channel epilogues, which replicate the
reference's fp32 arithmetic.

Spatial layout: each 28x28 image is zero-padded to 30x30 and flattened, so
every 3x3 conv tap is a pure diagonal shift in the flat index -> conv =
9 accumulating matmuls over contiguous windows.  We compute 30-wide output
rows (2 garbage columns per row) and discard the garbage in the epilogue APs.

Epilogue 1 (conv1 -> conv2 input):  q2 = rne(clip(P1*(7*sx*sw1*inv1) + 7*b1, +-7))
using the fp32 magic-number trick (+-1.5*2^23) for round-to-nearest-even;
the result is an exact integer written directly as fp8 into the padded conv2
input buffer.  The activation fake-quant scale alpha2 = max|hardtanh(...)| is
1.0 whenever anything clips (always, for this distribution); the kernel
computes max|.| on device and the host verifies it is exactly 7.0, falling
back to an exact numpy implementation otherwise.

Epilogue 2: y = clip(P2*(s2*sw2*inv2) + (x*inv2 + b2), +-1); the residual
affine x*inv2+b2 is precomputed on the host, so the device does one fused
scalar_tensor_tensor (scale+add) plus the clip on VectorE.

Inputs stream in four chained DMA stages ordered by first use (conv1/cot0
weights + images 0-1 first) so the PE starts ~11us into the kernel; the
measured span is ~125us, with the tensor engine at its DoubleRow streaming
floor (~105us, zero stalls) and fixed preamble/exit-barrier around it.
"""

import numpy as np
import ml_dtypes

EPS = np.float32(1e-5)
NCORES = 8
B, C, H, W = 64, 256, 28, 28
BC = B // NCORES            # images per core
IMS = 912                   # padded (30x30=900) image stride, multiple of 16
NT = 420                    # matmul N: 14 padded rows x 30
MAGIC = np.float32(12582912.0)  # 1.5 * 2^23
F8NP = ml_dtypes.float8_e4m3

WB = 36 * 2 * 128                   # 9216 bytes/partition of int weights
WG = 2 * 128                        # one weight group (tap)
VB = 48                             # 40B of fp32 epilogue vecs + 8B pad
IMB = 2 * IMS                       # one image (both channel halves)
X1B = BC * IMB                      # 14592 bytes/partition of int inputs
# staged layout: [w(conv1,cot0) | vec | imgs0-1 | w rest | imgs2-7]
W0B = 9 * WG
VOFF = W0B
X0OFF = W0B + VB
WROFF = X0OFF + 2 * IMB
X2OFF = WROFF + (WB - W0B)
INPB = X2OFF + 6 * IMB

_BUILT = None  # cached (nc,) so repeat calls skip IR building + compile


# ----------------------------------------------------------------- host math
def _quant_int(v):
    """Exact replica of the reference fake_quant grid; returns integer part."""
    alpha = np.float32(np.float32(np.max(np.abs(v))) + np.float32(1e-12))
    scale = np.float32(alpha / np.float32(7.0))
    q = np.round(np.clip(v, -alpha, alpha) / scale).astype(np.float32)
    return q, scale


def _fold_bn(gamma, beta, mean, var):
    gamma = np.asarray(gamma, np.float32)
    beta = np.asarray(beta, np.float32)
    mean = np.asarray(mean, np.float32)
    var = np.asarray(var, np.float32)
    inv = (gamma / np.sqrt(var + EPS)).astype(np.float32)
    b = (beta - mean * inv).astype(np.float32)
    return inv, b


# ------------------------------------------------------------------ bass IR
def _build():
    global _BUILT
    if _BUILT is not None:
        return _BUILT
    import concourse.bacc as bacc
    import concourse.tile as tile
    from concourse import mybir
    from contextlib import ExitStack

    f32 = mybir.dt.float32
    f8 = mybir.dt.float8e4
    AF = mybir.ActivationFunctionType
    OP = mybir.AluOpType
    DR = mybir.MatmulPerfMode.DoubleRow
    AX = mybir.AxisListType

    SA1, SA, SB1 = X0OFF + IMB, WROFF, X2OFF + 2 * IMB
    nc = bacc.Bacc("TRN2", target_bir_lowering=False, debug=False)
    inpa_d = nc.dram_tensor("inpa", [128, SA1], f8, kind="ExternalInput").ap()
    inpa2_d = nc.dram_tensor("inpa2", [128, SA - SA1], f8, kind="ExternalInput").ap()
    inpb1_d = nc.dram_tensor("inpb1", [128, SB1 - SA], f8, kind="ExternalInput").ap()
    inpb2_d = nc.dram_tensor("inpb2", [128, INPB - SB1], f8, kind="ExternalInput").ap()
    r_d = nc.dram_tensor("resid", [128, 2, BC, 2, 14, 28], f32, kind="ExternalInput").ap()
    y_d = nc.dram_tensor("y", [2, 128, BC, 2, 14, 28], f32, kind="ExternalOutput").ap()
    am_d = nc.dram_tensor("amax", [128, 4], f32, kind="ExternalOutput").ap()

    with tile.TileContext(nc) as tc, ExitStack() as ctx:
        const = ctx.enter_context(tc.tile_pool(name="const", bufs=1))
        psum = ctx.enter_context(tc.tile_pool(name="psum", bufs=8, space="PSUM"))
        ep1 = ctx.enter_context(tc.tile_pool(name="ep1", bufs=4))
        ep2 = ctx.enter_context(tc.tile_pool(name="ep2", bufs=4))
        yp = ctx.enter_context(tc.tile_pool(name="yp", bufs=3))

        inp_sb = const.tile([128, INPB], f8, tag="inp")
        x2_sb = const.tile([128, BC, 2, IMS], f8, tag="x2")
        rs_sb = const.tile([128, 2, BC, 2, 14, 28], f32, tag="rs")
        am_sb = const.tile([128, 4], f32, tag="am")

        vecv = inp_sb[:, VOFF:VOFF + 40].bitcast(f32)      # [128, 10] f32

        def w_ap(g):   # [128, 2, 128] tap g in (ci,cot)-major order
            off = g * WG if g < 9 else WROFF + (g - 9) * WG
            return inp_sb[:, off:off + WG].rearrange("p (r m) -> p r m", r=2)

        def x1_ap(b):  # [128, 2, IMS] image b
            off = X0OFF + b * IMB if b < 2 else X2OFF + (b - 2) * IMB
            return inp_sb[:, off:off + IMB].rearrange("p (r s) -> p r s", r=2)

        # HAM pre-warm: run junk matmuls on zeroed SBUF during the input-DMA
        # window so the PE clock gate is already at 2.4GHz (warm) when the
        # first real matmul issues (~3.4us of sustained activity required).
        wj = const.tile([128, 256], f8, tag="wj")
        nc.vector.memset(wj[:], 0.0)
        jl = wj[:].rearrange("p (r m) -> p r m", r=2)
        jp = psum.tile([128, NT], f32, tag="pt", name="jp")
        for _ in range(38):
            nc.tensor.matmul(jp[:, 0:128], jl, jl, start=True, stop=True,
                             perf_mode=DR)

        from concourse.tile_rust import add_dep_helper
        dma_a = nc.sync.dma_start(inp_sb[:, 0:SA1], inpa_d)
        dma_a2 = nc.sync.dma_start(inp_sb[:, SA1:SA], inpa2_d)
        dma_b1 = nc.sync.dma_start(inp_sb[:, SA:SB1], inpb1_d)
        dma_b2 = nc.sync.dma_start(inp_sb[:, SB1:INPB], inpb2_d)
        dma_r = nc.sync.dma_start(rs_sb[:], r_d)
        for a, b in ((dma_b1, dma_a2), (dma_b2, dma_b1), (dma_r, dma_b2)):
            add_dep_helper(a.ins, b.ins, sync=True,
                           reason="stage input DMAs by first-use order")
        nc.gpsimd.memset(x2_sb[:], 0.0)

        def vcol(i):
            return vecv[:, i : i + 1]

        def valid(ap420):  # [128,420] -> [128,14,28] dropping 2 garbage cols/row
            return ap420.rearrange("p (h w) -> p h w", w=30)[:, :, :28]

        for ci, src in ((0, None), (1, x2_sb)):
            # conv2 tapers to 1-image phases so the final epilogue tail is short
            groups = ([(b0, 2) for b0 in range(0, BC, 2)] if ci == 0 else
                      [(0, 2), (2, 2), (4, 2), (6, 1), (7, 1)])
            for b0, gsz in groups:
                for cot in range(2):
                    pts = {}
                    # -- 9 taps x (gsz images x 2 row-halves) per weight --
                    for k in range(9):
                        off = (k // 3) * 30 + (k % 3)
                        lhsT = w_ap((ci * 2 + cot) * 9 + k)
                        for bb in range(gsz):
                            b = b0 + bb
                            for hb in range(2):
                                if k == 0:
                                    pts[(bb, hb)] = psum.tile(
                                        [128, NT], f32, tag="pt", name="pt")
                                s = hb * NT + off
                                rhs = (x1_ap(b) if ci == 0 else
                                       src[:, b, :, :])[:, :, s : s + NT]
                                nc.tensor.matmul(
                                    pts[(bb, hb)][:], lhsT, rhs,
                                    start=(k == 0), stop=(k == 8), perf_mode=DR)
                    # ---- epilogues for this phase's psum tiles ----
                    for bb in range(gsz):
                        b = b0 + bb
                        if ci == 1:
                            yb = yp.tile([128, 2, 14, 28], f32, tag="yb", name="yb")
                        for hb in range(2):
                            pt3 = valid(pts[(bb, hb)][:])
                            if ci == 0:
                                # t=P*a1+b1p ; clip +-7 ; +-MAGIC rne -> fp8
                                t1 = ep1.tile([128, 14, 28], f32, tag="t1", name="t1")
                                nc.scalar.activation(
                                    t1[:], pt3, AF.Identity,
                                    bias=vcol(2 + cot), scale=vcol(0 + cot))
                                t2 = ep1.tile([128, 14, 28], f32, tag="t2", name="t2")
                                nc.vector.tensor_scalar(
                                    t2[:], t1[:], 7.0, -7.0, op0=OP.min, op1=OP.max)
                                if bb == 0 and hb == 0 and b0 in (0, 2):
                                    # any tile hitting exactly 7.0 proves
                                    # alpha2 == 1.0 globally (clip bound)
                                    idx = (b0 // 2) * 2 + cot
                                    nc.vector.tensor_reduce(
                                        am_sb[:, idx : idx + 1], t2[:], op=OP.max,
                                        axis=AX.XY, apply_absolute_value=True)
                                t3 = ep1.tile([128, 14, 28], f32, tag="t3", name="t3")
                                nc.scalar.activation(
                                    t3[:], t2[:], AF.Copy, bias=float(MAGIC), scale=1.0)
                                dst = valid(
                                    x2_sb[:, b, cot, hb * NT + 31 : hb * NT + 31 + NT])
                                nc.vector.tensor_scalar(
                                    dst, t3[:], -float(MAGIC), None, op0=OP.add)
                            else:
                                # y = clip(P2*c2 + (x*inv2 + b2), +-1);
                                # the residual affine is precomputed on host
                                u3 = ep2.tile([128, 14, 28], f32, tag="u3", name="u3")
                                nc.vector.scalar_tensor_tensor(
                                    u3[:], pt3, vcol(4 + cot), rs_sb[:, cot, b, hb],
                                    op0=OP.mult, op1=OP.add)
                                nc.vector.tensor_scalar(
                                    yb[:, hb], u3[:], 1.0, -1.0,
                                    op0=OP.min, op1=OP.max)
                                if gsz == 1:
                                    nc.sync.dma_start(y_d[cot, :, b, hb],
                                                      yb[:, hb])
                        if ci == 1 and gsz > 1:
                            nc.sync.dma_start(y_d[cot, :, b], yb[:])
            if ci == 0:
                nc.sync.dma_start(am_d, am_sb[:])

    nc.compile()
    _dedupe_ldweights(nc)
    _BUILT = (nc,)
    return _BUILT


# ------------------------------------------------------------- input packing
def _prep(x, w1, w2, inv1, b1, inv2, b2):
    xi, s_x = _quant_int(x)
    w1i, s_w1 = _quant_int(w1)
    w2i, s_w2 = _quant_int(w2)

    xi8 = xi.astype(F8NP)
    tmp = np.zeros((NCORES, BC, 2, 128, 30, 30), F8NP)
    tmp[:, :, :, :, 1:29, 1:29] = xi8.reshape(NCORES, BC, 2, 128, 28, 28)
    x1_all = np.zeros((NCORES, 128, BC, 2, IMS), F8NP)
    x1_all[..., :900] = tmp.transpose(0, 3, 1, 2, 4, 5).reshape(
        NCORES, 128, BC, 2, 900)

    def wpack(wi):
        # w[cot*128+m, r*128+p, kh, kw] -> [p, (cot,k), r, m]
        v = wi.reshape(2, 128, 2, 128, 9)          # cot, m, r, p, k
        v = v.transpose(3, 0, 4, 2, 1)             # p, cot, k, r, m
        return v.reshape(128, 18, 2, 128).astype(F8NP)

    w_all = np.concatenate([wpack(w1i), wpack(w2i)], axis=1).reshape(128, WB)

    s2 = np.float32(np.float32(1.0) / np.float32(7.0))
    a1 = (np.float32(7.0) * s_x * s_w1 * inv1).astype(np.float32)
    b1p = (np.float32(7.0) * b1).astype(np.float32)
    c2 = (s2 * s_w2 * inv2).astype(np.float32)
    cols = [a1[:128], a1[128:], b1p[:128], b1p[128:], c2[:128], c2[128:],
            inv2[:128], inv2[128:], b2[:128], b2[128:]]
    vec8 = np.zeros((128, VB), F8NP)
    vec8[:, :40] = np.ascontiguousarray(
        np.stack(cols, axis=1).astype(np.float32)).view(F8NP)

    # residual affine x*inv2 + b2, precomputed -> [cores, 128(m), 2(cot), BC, ...]
    rs2 = (x * inv2[None, :, None, None] + b2[None, :, None, None]).astype(np.float32)
    resid = rs2.reshape(NCORES, BC, 2, 128, 2, 14, 28).transpose(0, 3, 2, 1, 4, 5, 6)
    resid = np.ascontiguousarray(resid)

    in_maps = []
    for i in range(NCORES):
        x1i = x1_all[i].reshape(128, X1B)
        inpa = np.concatenate([w_all[:, :W0B], vec8, x1i[:, :IMB]], axis=1)
        inpa2 = x1i[:, IMB:2 * IMB]
        inpb1 = np.concatenate([w_all[:, W0B:], x1i[:, 2 * IMB:4 * IMB]], axis=1)
        inpb2 = x1i[:, 4 * IMB:]
        in_maps.append({"inpa": np.ascontiguousarray(inpa),
                        "inpa2": np.ascontiguousarray(inpa2),
                        "inpb1": np.ascontiguousarray(inpb1),
                        "inpb2": np.ascontiguousarray(inpb2),
                        "resid": resid[i]})
    return in_maps, (xi, w1i, w2i, s_x, s_w1, s_w2, s2)


# ------------------------------------------------------- exact numpy fallback
def _conv3x3_int(xint, wint):
    Bn, Cn, Hn, Wn = xint.shape
    xp = np.zeros((Bn, Cn, Hn + 2, Wn + 2), np.float64)
    xp[:, :, 1:-1, 1:-1] = xint
    out = np.zeros((Bn, wint.shape[0], Hn, Wn), np.float64)
    w64 = wint.astype(np.float64)
    for kh in range(3):
        for kw in range(3):
            out += np.einsum("bchw,oc->bohw", xp[:, :, kh:kh + Hn, kw:kw + Wn],
                             w64[:, :, kh, kw], optimize=True)
    return out.astype(np.float32)


def _numpy_path(x, q, inv1, b1, inv2, b2):
    """Exact replica handling arbitrary alpha2 (never expected to run)."""
    xi, w1i, w2i, s_x, s_w1, s_w2, _ = q
    P1 = _conv3x3_int(xi, w1i)
    h = (P1 * (s_x * s_w1 * inv1)[None, :, None, None]).astype(np.float32)
    h = (h + b1[None, :, None, None]).astype(np.float32)
    h = np.clip(h, np.float32(-1.0), np.float32(1.0))
    alpha2 = np.float32(np.abs(h).max())
    s2 = np.float32(alpha2 / np.float32(7.0))
    x2 = np.round(np.clip(h, -alpha2, alpha2) / s2).astype(np.float32)
    P2 = _conv3x3_int(x2, w2i)
    u = (P2 * (s2 * s_w2 * inv2)[None, :, None, None]).astype(np.float32)
    u = (u + (x * inv2[None, :, None, None] + b2[None, :, None, None])).astype(np.float32)
    return np.clip(u, np.float32(-1.0), np.float32(1.0))


# ------------------------------------------------------------------- kernel
def _dedupe_ldweights(nc):
    """Drop InstLdweights that reload the stationary operand already in the
    PE array (consecutive matmuls here reuse one weight 8x).  Safe because
    Ldweights carry no semaphore updates; ones carrying waits are kept."""
    for f in nc.m.functions:
        for blk in f.blocks:
            il = blk.instructions
            keep, last_sig, removed = [], None, 0
            for ins in il:
                tn = type(ins).__name__
                if tn == "InstLdweights":
                    sig = (str(ins.ins), str(ins.perf_mode),
                           str(ins.tile_position), str(ins.is_transpose))
                    plain = ("wait:" not in str(ins)
                             and "update:" not in str(ins))
                    if sig == last_sig and plain:
                        removed += 1
                        continue
                    last_sig = sig
                elif tn in ("InstMatmult", "InstEventSemaphore", "InstDrain"):
                    pass                     # none of these clobber loaded weights
                elif str(getattr(ins, "engine", "")).endswith("PE"):
                    last_sig = None          # conservative reset on other PE ops
                keep.append(ins)
            if removed:
                il[:] = keep


def _run(in_maps, trace=False, tmpdir=None):
    from concourse.bass_utils import run_bass_kernel_spmd
    (nc,) = _build()
    return run_bass_kernel_spmd(nc, in_maps, list(range(NCORES)), trace=trace,
                                tmpdir=tmpdir)


def kernel(x, w1, bn1_gamma, bn1_beta, bn1_mean, bn1_var,
           w2, bn2_gamma, bn2_beta, bn2_mean, bn2_var):
    x = np.asarray(x, np.float32)
    w1 = np.asarray(w1, np.float32)
    w2 = np.asarray(w2, np.float32)
    inv1, b1 = _fold_bn(bn1_gamma, bn1_beta, bn1_mean, bn1_var)
    inv2, b2 = _fold_bn(bn2_gamma, bn2_beta, bn2_mean, bn2_var)

    in_maps, q = _prep(x, w1, w2, inv1, b1, inv2, b2)
    res = _run(in_maps)

    amax = np.max([r["amax"] for r in res.results])
    if not np.float32(amax) == np.float32(7.0):
        return _numpy_path(x, q, inv1, b1, inv2, b2)

    ys = np.stack([r["y"] for r in res.results])      # [cores, 2, 128, BC, 2,14,28]
    ys = ys.reshape(NCORES, 2, 128, BC, 784)
    return ys.transpose(0, 3, 1, 2, 4).reshape(B, C, H, W).copy()


# revision 25
# speedup vs baseline: 1.0089x; 1.0040x over previous
"""Trainium2 Bass kernel for the quantized BasicBlock (nn_BasicBlock_15436112462307).

Strategy
--------
Data-parallel over batch: 64 images -> 8 cores x 8 images. Weights/BN replicated.

fake_quant makes every conv operand an exact small integer (-7..7) times a
global fp32 scale.  We factor the scales out on the host and feed pure
integers to the PE as fp8e4 (integers <=7 are exact in fp8e4), using
perf_mode=DoubleRow so one matmul contracts all 256 input channels
(lhsT [128,2,128] / rhs [128,2,N]) at 2x fp8 rate.  PSUM accumulates the
integer dot products exactly in fp32, so the conv itself is EXACT; all
rounding happens only in the per-channel epilogues, which replicate the
reference's fp32 arithmetic.

Spatial layout: each 28x28 image is zero-padded to 30x30 and flattened, so
every 3x3 conv tap is a pure diagonal shift in the flat index -> conv =
9 accumulating matmuls over contiguous windows.  We compute 30-wide output
rows (2 garbage columns per row) and discard the garbage in the epilogue APs.

Epilogue 1 (conv1 -> conv2 input):  q2 = rne(clip(P1*(7*sx*sw1*inv1) + 7*b1, +-7))
using the fp32 magic-number trick (+-1.5*2^23) for round-to-nearest-even;
the result is an exact integer written directly as fp8 into the padded conv2
input buffer.  The activation fake-quant scale alpha2 = max|hardtanh(...)| is
1.0 whenever anything clips (always, for this distribution); the kernel
computes max|.| on device and the host verifies it is exactly 7.0, falling
back to an exact numpy implementation otherwise.

Epilogue 2: y = clip(P2*(s2*sw2*inv2) + (x*inv2 + b2), +-1); the residual
affine x*inv2+b2 is precomputed on the host, so the device does one fused
scalar_tensor_tensor (scale+add) plus the clip on VectorE.

Inputs stream in staged DMAs ordered by first use (conv1/cot0 weights +
images 0-1 in two parallel transfers, the rest chained behind).  Junk
matmuls on zeroed SBUF fill the DMA wait so the PE HAM clock-gate is warm
(2.4GHz) when real matmuls start ~11us in.  Measured ~121.5us: the tensor
engine runs its 576 DoubleRow matmuls at the streaming floor (~103us,
zero stalls, one HAM transition), conv2 tapers to 1-image phases so the
epilogue tail is ~2us, and the rest is fixed preamble + exit barrier.
"""

import numpy as np
import ml_dtypes

EPS = np.float32(1e-5)
NCORES = 8
B, C, H, W = 64, 256, 28, 28
BC = B // NCORES            # images per core
IMS = 912                   # padded (30x30=900) image stride, multiple of 16
NT = 420                    # matmul N: 14 padded rows x 30
MAGIC = np.float32(12582912.0)  # 1.5 * 2^23
F8NP = ml_dtypes.float8_e4m3

WB = 36 * 2 * 128                   # 9216 bytes/partition of int weights
WG = 2 * 128                        # one weight group (tap)
VB = 48                             # 40B of fp32 epilogue vecs + 8B pad
IMB = 2 * IMS                       # one image (both channel halves)
X1B = BC * IMB                      # 14592 bytes/partition of int inputs
# staged layout: [w(conv1,cot0) | vec | imgs0-1 | w rest | imgs2-7]
W0B = 9 * WG
VOFF = W0B
X0OFF = W0B + VB
WROFF = X0OFF + 2 * IMB
X2OFF = WROFF + (WB - W0B)
INPB = X2OFF + 6 * IMB

_BUILT = None  # cached (nc,) so repeat calls skip IR building + compile


# ----------------------------------------------------------------- host math
def _quant_int(v):
    """Exact replica of the reference fake_quant grid; returns integer part."""
    alpha = np.float32(np.float32(np.max(np.abs(v))) + np.float32(1e-12))
    scale = np.float32(alpha / np.float32(7.0))
    q = np.round(np.clip(v, -alpha, alpha) / scale).astype(np.float32)
    return q, scale


def _fold_bn(gamma, beta, mean, var):
    gamma = np.asarray(gamma, np.float32)
    beta = np.asarray(beta, np.float32)
    mean = np.asarray(mean, np.float32)
    var = np.asarray(var, np.float32)
    inv = (gamma / np.sqrt(var + EPS)).astype(np.float32)
    b = (beta - mean * inv).astype(np.float32)
    return inv, b


# ------------------------------------------------------------------ bass IR
def _build():
    global _BUILT
    if _BUILT is not None:
        return _BUILT
    import concourse.bacc as bacc
    import concourse.tile as tile
    from concourse import mybir
    from contextlib import ExitStack

    f32 = mybir.dt.float32
    f8 = mybir.dt.float8e4
    AF = mybir.ActivationFunctionType
    OP = mybir.AluOpType
    DR = mybir.MatmulPerfMode.DoubleRow
    AX = mybir.AxisListType

    SA1, SA, SB1 = X0OFF + IMB, WROFF, X2OFF + 2 * IMB
    nc = bacc.Bacc("TRN2", target_bir_lowering=False, debug=False)
    inpa_d = nc.dram_tensor("inpa", [128, SA1], f8, kind="ExternalInput").ap()
    inpa2_d = nc.dram_tensor("inpa2", [128, SA - SA1], f8, kind="ExternalInput").ap()
    inpb1_d = nc.dram_tensor("inpb1", [128, SB1 - SA], f8, kind="ExternalInput").ap()
    inpb2_d = nc.dram_tensor("inpb2", [128, INPB - SB1], f8, kind="ExternalInput").ap()
    r_d = nc.dram_tensor("resid", [128, 2, BC, 2, 14, 28], f32, kind="ExternalInput").ap()
    y_d = nc.dram_tensor("y", [2, 128, BC, 2, 14, 28], f32, kind="ExternalOutput").ap()
    am_d = nc.dram_tensor("amax", [128, 4], f32, kind="ExternalOutput").ap()

    with tile.TileContext(nc) as tc, ExitStack() as ctx:
        const = ctx.enter_context(tc.tile_pool(name="const", bufs=1))
        psum = ctx.enter_context(tc.tile_pool(name="psum", bufs=8, space="PSUM"))
        ep1 = ctx.enter_context(tc.tile_pool(name="ep1", bufs=4))
        ep2 = ctx.enter_context(tc.tile_pool(name="ep2", bufs=4))
        yp = ctx.enter_context(tc.tile_pool(name="yp", bufs=3))

        inp_sb = const.tile([128, INPB], f8, tag="inp")
        x2_sb = const.tile([128, BC, 2, IMS], f8, tag="x2")
        rs_sb = const.tile([128, 2, BC, 2, 14, 28], f32, tag="rs")
        am_sb = const.tile([128, 4], f32, tag="am")

        vecv = inp_sb[:, VOFF:VOFF + 40].bitcast(f32)      # [128, 10] f32

        def w_ap(g):   # [128, 2, 128] tap g in (ci,cot)-major order
            off = g * WG if g < 9 else WROFF + (g - 9) * WG
            return inp_sb[:, off:off + WG].rearrange("p (r m) -> p r m", r=2)

        def x1_ap(b):  # [128, 2, IMS] image b
            off = X0OFF + b * IMB if b < 2 else X2OFF + (b - 2) * IMB
            return inp_sb[:, off:off + IMB].rearrange("p (r s) -> p r s", r=2)

        # HAM pre-warm: run junk matmuls on zeroed SBUF during the input-DMA
        # window so the PE clock gate is already at 2.4GHz (warm) when the
        # first real matmul issues (~3.4us of sustained activity required).
        wj = const.tile([128, 256], f8, tag="wj")
        nc.vector.memset(wj[:], 0.0)
        jl = wj[:].rearrange("p (r m) -> p r m", r=2)
        jp = psum.tile([128, NT], f32, tag="pt", name="jp")
        for _ in range(38):
            nc.tensor.matmul(jp[:, 0:128], jl, jl, start=True, stop=True,
                             perf_mode=DR)

        from concourse.tile_rust import add_dep_helper
        dma_a = nc.sync.dma_start(inp_sb[:, 0:SA1], inpa_d)
        dma_a2 = nc.sync.dma_start(inp_sb[:, SA1:SA], inpa2_d)
        dma_b1 = nc.sync.dma_start(inp_sb[:, SA:SB1], inpb1_d)
        dma_b2 = nc.sync.dma_start(inp_sb[:, SB1:INPB], inpb2_d)
        dma_r = nc.sync.dma_start(rs_sb[:], r_d)
        for a, b in ((dma_b1, dma_a2), (dma_b2, dma_b1), (dma_r, dma_b2)):
            add_dep_helper(a.ins, b.ins, sync=True,
                           reason="stage input DMAs by first-use order")
        nc.gpsimd.memset(x2_sb[:], 0.0)

        def vcol(i):
            return vecv[:, i : i + 1]

        def valid(ap420):  # [128,420] -> [128,14,28] dropping 2 garbage cols/row
            return ap420.rearrange("p (h w) -> p h w", w=30)[:, :, :28]

        for ci, src in ((0, None), (1, x2_sb)):
            # conv2 tapers to 1-image phases so the final epilogue tail is short
            groups = ([(b0, 2) for b0 in range(0, BC, 2)] if ci == 0 else
                      [(0, 2), (2, 2), (4, 2), (6, 1), (7, 1)])
            for b0, gsz in groups:
                for cot in range(2):
                    pts = {}
                    # -- 9 taps x (gsz images x 2 row-halves) per weight --
                    for k in range(9):
                        off = (k // 3) * 30 + (k % 3)
                        lhsT = w_ap((ci * 2 + cot) * 9 + k)
                        for bb in range(gsz):
                            b = b0 + bb
                            for hb in range(2):
                                if k == 0:
                                    pts[(bb, hb)] = psum.tile(
                                        [128, NT], f32, tag="pt", name="pt")
                                s = hb * NT + off
                                rhs = (x1_ap(b) if ci == 0 else
                                       src[:, b, :, :])[:, :, s : s + NT]
                                nc.tensor.matmul(
                                    pts[(bb, hb)][:], lhsT, rhs,
                                    start=(k == 0), stop=(k == 8), perf_mode=DR)
                    # ---- epilogues for this phase's psum tiles ----
                    for bb in range(gsz):
                        b = b0 + bb
                        if ci == 1:
                            yb = yp.tile([128, 2, 14, 28], f32, tag="yb", name="yb")
                        for hb in range(2):
                            pt3 = valid(pts[(bb, hb)][:])
                            if ci == 0:
                                # t=P*a1+b1p ; clip +-7 ; +-MAGIC rne -> fp8
                                t1 = ep1.tile([128, 14, 28], f32, tag="t1", name="t1")
                                nc.scalar.activation(
                                    t1[:], pt3, AF.Identity,
                                    bias=vcol(2 + cot), scale=vcol(0 + cot))
                                t2 = ep1.tile([128, 14, 28], f32, tag="t2", name="t2")
                                nc.vector.tensor_scalar(
                                    t2[:], t1[:], 7.0, -7.0, op0=OP.min, op1=OP.max)
                                if bb == 0 and hb == 0 and b0 in (0, 2):
                                    # any tile hitting exactly 7.0 proves
                                    # alpha2 == 1.0 globally (clip bound)
                                    idx = (b0 // 2) * 2 + cot
                                    nc.vector.tensor_reduce(
                                        am_sb[:, idx : idx + 1], t2[:], op=OP.max,
                                        axis=AX.XY, apply_absolute_value=True)
                                t3 = ep1.tile([128, 14, 28], f32, tag="t3", name="t3")
                                nc.scalar.activation(
                                    t3[:], t2[:], AF.Copy, bias=float(MAGIC), scale=1.0)
                                dst = valid(
                                    x2_sb[:, b, cot, hb * NT + 31 : hb * NT + 31 + NT])
                                nc.vector.tensor_scalar(
                                    dst, t3[:], -float(MAGIC), None, op0=OP.add)
                            else:
                                # y = clip(P2*c2 + (x*inv2 + b2), +-1);
                                # the residual affine is precomputed on host
                                u3 = ep2.tile([128, 14, 28], f32, tag="u3", name="u3")
                                nc.vector.scalar_tensor_tensor(
                                    u3[:], pt3, vcol(4 + cot), rs_sb[:, cot, b, hb],
                                    op0=OP.mult, op1=OP.add)
                                nc.vector.tensor_scalar(
                                    yb[:, hb], u3[:], 1.0, -1.0,
                                    op0=OP.min, op1=OP.max)
                                if gsz == 1:
                                    nc.sync.dma_start(y_d[cot, :, b, hb],
                                                      yb[:, hb])
                        if ci == 1 and gsz > 1:
                            nc.sync.dma_start(y_d[cot, :, b], yb[:])
            if ci == 0:
                nc.sync.dma_start(am_d, am_sb[:])

    nc.compile()
    _dedupe_ldweights(nc)
    _BUILT = (nc,)
    return _BUILT


# ------------------------------------------------------------- input packing
def _prep(x, w1, w2, inv1, b1, inv2, b2):
    xi, s_x = _quant_int(x)
    w1i, s_w1 = _quant_int(w1)
    w2i, s_w2 = _quant_int(w2)

    xi8 = xi.astype(F8NP)
    tmp = np.zeros((NCORES, BC, 2, 128, 30, 30), F8NP)
    tmp[:, :, :, :, 1:29, 1:29] = xi8.reshape(NCORES, BC, 2, 128, 28, 28)
    x1_all = np.zeros((NCORES, 128, BC, 2, IMS), F8NP)
    x1_all[..., :900] = tmp.transpose(0, 3, 1, 2, 4, 5).reshape(
        NCORES, 128, BC, 2, 900)

    def wpack(wi):
        # w[cot*128+m, r*128+p, kh, kw] -> [p, (cot,k), r, m]
        v = wi.reshape(2, 128, 2, 128, 9)          # cot, m, r, p, k
        v = v.transpose(3, 0, 4, 2, 1)             # p, cot, k, r, m
        return v.reshape(128, 18, 2, 128).astype(F8NP)

    w_all = np.concatenate([wpack(w1i), wpack(w2i)], axis=1).reshape(128, WB)

    s2 = np.float32(np.float32(1.0) / np.float32(7.0))
    a1 = (np.float32(7.0) * s_x * s_w1 * inv1).astype(np.float32)
    b1p = (np.float32(7.0) * b1).astype(np.float32)
    c2 = (s2 * s_w2 * inv2).astype(np.float32)
    cols = [a1[:128], a1[128:], b1p[:128], b1p[128:], c2[:128], c2[128:],
            inv2[:128], inv2[128:], b2[:128], b2[128:]]
    vec8 = np.zeros((128, VB), F8NP)
    vec8[:, :40] = np.ascontiguousarray(
        np.stack(cols, axis=1).astype(np.float32)).view(F8NP)

    # residual affine x*inv2 + b2, precomputed -> [cores, 128(m), 2(cot), BC, ...]
    rs2 = (x * inv2[None, :, None, None] + b2[None, :, None, None]).astype(np.float32)
    resid = rs2.reshape(NCORES, BC, 2, 128, 2, 14, 28).transpose(0, 3, 2, 1, 4, 5, 6)
    resid = np.ascontiguousarray(resid)

    in_maps = []
    for i in range(NCORES):
        x1i = x1_all[i].reshape(128, X1B)
        inpa = np.concatenate([w_all[:, :W0B], vec8, x1i[:, :IMB]], axis=1)
        inpa2 = x1i[:, IMB:2 * IMB]
        inpb1 = np.concatenate([w_all[:, W0B:], x1i[:, 2 * IMB:4 * IMB]], axis=1)
        inpb2 = x1i[:, 4 * IMB:]
        in_maps.append({"inpa": np.ascontiguousarray(inpa),
                        "inpa2": np.ascontiguousarray(inpa2),
                        "inpb1": np.ascontiguousarray(inpb1),
                        "inpb2": np.ascontiguousarray(inpb2),
                        "resid": resid[i]})
    return in_maps, (xi, w1i, w2i, s_x, s_w1, s_w2, s2)


# ------------------------------------------------------- exact numpy fallback
def _conv3x3_int(xint, wint):
    Bn, Cn, Hn, Wn = xint.shape
    xp = np.zeros((Bn, Cn, Hn + 2, Wn + 2), np.float64)
    xp[:, :, 1:-1, 1:-1] = xint
    out = np.zeros((Bn, wint.shape[0], Hn, Wn), np.float64)
    w64 = wint.astype(np.float64)
    for kh in range(3):
        for kw in range(3):
            out += np.einsum("bchw,oc->bohw", xp[:, :, kh:kh + Hn, kw:kw + Wn],
                             w64[:, :, kh, kw], optimize=True)
    return out.astype(np.float32)


def _numpy_path(x, q, inv1, b1, inv2, b2):
    """Exact replica handling arbitrary alpha2 (never expected to run)."""
    xi, w1i, w2i, s_x, s_w1, s_w2, _ = q
    P1 = _conv3x3_int(xi, w1i)
    h = (P1 * (s_x * s_w1 * inv1)[None, :, None, None]).astype(np.float32)
    h = (h + b1[None, :, None, None]).astype(np.float32)
    h = np.clip(h, np.float32(-1.0), np.float32(1.0))
    alpha2 = np.float32(np.abs(h).max())
    s2 = np.float32(alpha2 / np.float32(7.0))
    x2 = np.round(np.clip(h, -alpha2, alpha2) / s2).astype(np.float32)
    P2 = _conv3x3_int(x2, w2i)
    u = (P2 * (s2 * s_w2 * inv2)[None, :, None, None]).astype(np.float32)
    u = (u + (x * inv2[None, :, None, None] + b2[None, :, None, None])).astype(np.float32)
    return np.clip(u, np.float32(-1.0), np.float32(1.0))


# ------------------------------------------------------------------- kernel
def _dedupe_ldweights(nc):
    """Drop InstLdweights that reload the stationary operand already in the
    PE array (consecutive matmuls here reuse one weight 8x).  Safe because
    Ldweights carry no semaphore updates; ones carrying waits are kept."""
    for f in nc.m.functions:
        for blk in f.blocks:
            il = blk.instructions
            keep, last_sig, removed = [], None, 0
            for ins in il:
                tn = type(ins).__name__
                if tn == "InstLdweights":
                    sig = (str(ins.ins), str(ins.perf_mode),
                           str(ins.tile_position), str(ins.is_transpose))
                    plain = ("wait:" not in str(ins)
                             and "update:" not in str(ins))
                    if sig == last_sig and plain:
                        removed += 1
                        continue
                    last_sig = sig
                elif tn in ("InstMatmult", "InstEventSemaphore", "InstDrain"):
                    pass                     # none of these clobber loaded weights
                elif str(getattr(ins, "engine", "")).endswith("PE"):
                    last_sig = None          # conservative reset on other PE ops
                keep.append(ins)
            if removed:
                il[:] = keep


def _run(in_maps, trace=False, tmpdir=None):
    from concourse.bass_utils import run_bass_kernel_spmd
    (nc,) = _build()
    return run_bass_kernel_spmd(nc, in_maps, list(range(NCORES)), trace=trace,
                                tmpdir=tmpdir)


def kernel(x, w1, bn1_gamma, bn1_beta, bn1_mean, bn1_var,
           w2, bn2_gamma, bn2_beta, bn2_mean, bn2_var):
    x = np.asarray(x, np.float32)
    w1 = np.asarray(w1, np.float32)
    w2 = np.asarray(w2, np.float32)
    inv1, b1 = _fold_bn(bn1_gamma, bn1_beta, bn1_mean, bn1_var)
    inv2, b2 = _fold_bn(bn2_gamma, bn2_beta, bn2_mean, bn2_var)

    in_maps, q = _prep(x, w1, w2, inv1, b1, inv2, b2)
    res = _run(in_maps)

    amax = np.max([r["amax"] for r in res.results])
    if not np.float32(amax) == np.float32(7.0):
        return _numpy_path(x, q, inv1, b1, inv2, b2)

    ys = np.stack([r["y"] for r in res.results])      # [cores, 2, 128, BC, 2,14,28]
    ys = ys.reshape(NCORES, 2, 128, BC, 784)
    return ys.transpose(0, 3, 1, 2, 4).reshape(B, C, H, W).copy()


# revision 27
# speedup vs baseline: 1.0470x; 1.0378x over previous
"""Trainium2 Bass kernel for the quantized BasicBlock (nn_BasicBlock_15436112462307).

Strategy
--------
Data-parallel over batch: 64 images -> 8 cores x 8 images. Weights/BN replicated.

fake_quant makes every conv operand an exact small integer (-7..7) times a
global fp32 scale.  We factor the scales out on the host and feed pure
integers to the PE as fp8e4 (integers <=7 are exact in fp8e4), using
perf_mode=DoubleRow so one matmul contracts all 256 input channels
(lhsT [128,2,128] / rhs [128,2,N]) at 2x fp8 rate.  PSUM accumulates the
integer dot products exactly in fp32, so the conv itself is EXACT; all
rounding happens only in the per-channel epilogues, which replicate the
reference's fp32 arithmetic.

Spatial layout: each 28x28 image is zero-padded to 30x30 and flattened, so
every 3x3 conv tap is a pure diagonal shift in the flat index -> conv =
9 accumulating matmuls over contiguous windows.  We compute 30-wide output
rows (2 garbage columns per row) and discard the garbage in the epilogue APs.

Epilogue 1 (conv1 -> conv2 input):  q2 = rne(clip(P1*(7*sx*sw1*inv1) + 7*b1, +-7))
using the fp32 magic-number trick (+-1.5*2^23) for round-to-nearest-even;
the result is an exact integer written directly as fp8 into the padded conv2
input buffer.  The activation fake-quant scale alpha2 = max|hardtanh(...)| is
1.0 whenever anything clips (always, for this distribution); the kernel
computes max|.| on device and the host verifies it is exactly 7.0, falling
back to an exact numpy implementation otherwise.

Epilogue 2: y = clip(P2*(s2*sw2*inv2) + (x*inv2 + b2), +-1); the residual
affine x*inv2+b2 is precomputed on the host, so the device does one fused
scalar_tensor_tensor (scale+add) plus the clip on VectorE.

Inputs stream in staged DMAs ordered by first use (conv1/cot0 weights +
images 0-1 in two parallel transfers, the rest chained behind).  Junk
matmuls on zeroed SBUF fill the DMA wait so the PE HAM clock-gate is warm
(2.4GHz) when real matmuls start ~11us in.  Measured ~121.5us: the tensor
engine runs its 576 DoubleRow matmuls at the streaming floor (~103us,
zero stalls, one HAM transition), conv2 tapers to 1-image phases so the
epilogue tail is ~2us, and the rest is fixed preamble + exit barrier.
"""

import numpy as np
import ml_dtypes

EPS = np.float32(1e-5)
NCORES = 8
B, C, H, W = 64, 256, 28, 28
BC = B // NCORES            # images per core
IMS = 880                   # padded (30 rows x 29 cols = 870) image stride;
                            # one zero col shared as right-pad of row h and
                            # left-pad of row h+1
NT = 406                    # matmul N: 14 padded rows x 29
MAGIC = np.float32(12582912.0)  # 1.5 * 2^23
F8NP = ml_dtypes.float8_e4m3

WB = 36 * 2 * 128                   # 9216 bytes/partition of int weights
WG = 2 * 128                        # one weight group (tap)
VB = 48                             # 40B of fp32 epilogue vecs + 8B pad
IMB = 2 * IMS                       # one image (both channel halves)
X1B = BC * IMB                      # 14592 bytes/partition of int inputs
# staged layout: [w(conv1,cot0) | vec | imgs0-1 | w rest | imgs2-7]
W0B = 9 * WG
VOFF = W0B
X0OFF = W0B + VB
WROFF = X0OFF + 2 * IMB
X2OFF = WROFF + (WB - W0B)
INPB = X2OFF + 6 * IMB

_BUILT = None  # cached (nc,) so repeat calls skip IR building + compile


# ----------------------------------------------------------------- host math
def _quant_int(v):
    """Exact replica of the reference fake_quant grid; returns integer part."""
    alpha = np.float32(np.float32(np.max(np.abs(v))) + np.float32(1e-12))
    scale = np.float32(alpha / np.float32(7.0))
    q = np.round(np.clip(v, -alpha, alpha) / scale).astype(np.float32)
    return q, scale


def _fold_bn(gamma, beta, mean, var):
    gamma = np.asarray(gamma, np.float32)
    beta = np.asarray(beta, np.float32)
    mean = np.asarray(mean, np.float32)
    var = np.asarray(var, np.float32)
    inv = (gamma / np.sqrt(var + EPS)).astype(np.float32)
    b = (beta - mean * inv).astype(np.float32)
    return inv, b


# ------------------------------------------------------------------ bass IR
def _build():
    global _BUILT
    if _BUILT is not None:
        return _BUILT
    import concourse.bacc as bacc
    import concourse.tile as tile
    from concourse import mybir
    from contextlib import ExitStack

    f32 = mybir.dt.float32
    f8 = mybir.dt.float8e4
    AF = mybir.ActivationFunctionType
    OP = mybir.AluOpType
    DR = mybir.MatmulPerfMode.DoubleRow
    AX = mybir.AxisListType

    SA1, SA, SB1 = X0OFF + IMB, WROFF, X2OFF + 2 * IMB
    nc = bacc.Bacc("TRN2", target_bir_lowering=False, debug=False)
    inpa_d = nc.dram_tensor("inpa", [128, SA1], f8, kind="ExternalInput").ap()
    inpa2_d = nc.dram_tensor("inpa2", [128, SA - SA1], f8, kind="ExternalInput").ap()
    inpb1_d = nc.dram_tensor("inpb1", [128, SB1 - SA], f8, kind="ExternalInput").ap()
    inpb2_d = nc.dram_tensor("inpb2", [128, INPB - SB1], f8, kind="ExternalInput").ap()
    r_d = nc.dram_tensor("resid", [128, 2, BC, 2, 14, 28], f32, kind="ExternalInput").ap()
    y_d = nc.dram_tensor("y", [2, 128, BC, 2, 14, 28], f32, kind="ExternalOutput").ap()
    am_d = nc.dram_tensor("amax", [128, 4], f32, kind="ExternalOutput").ap()

    with tile.TileContext(nc) as tc, ExitStack() as ctx:
        const = ctx.enter_context(tc.tile_pool(name="const", bufs=1))
        psum = ctx.enter_context(tc.tile_pool(name="psum", bufs=8, space="PSUM"))
        ep1 = ctx.enter_context(tc.tile_pool(name="ep1", bufs=4))
        ep2 = ctx.enter_context(tc.tile_pool(name="ep2", bufs=4))
        yp = ctx.enter_context(tc.tile_pool(name="yp", bufs=3))

        inp_sb = const.tile([128, INPB], f8, tag="inp")
        x2_sb = const.tile([128, BC, 2, IMS], f8, tag="x2")
        rs_sb = const.tile([128, 2, BC, 2, 14, 28], f32, tag="rs")
        am_sb = const.tile([128, 4], f32, tag="am")

        vecv = inp_sb[:, VOFF:VOFF + 40].bitcast(f32)      # [128, 10] f32

        def w_ap(g):   # [128, 2, 128] tap g in (ci,cot)-major order
            off = g * WG if g < 9 else WROFF + (g - 9) * WG
            return inp_sb[:, off:off + WG].rearrange("p (r m) -> p r m", r=2)

        def x1_ap(b):  # [128, 2, IMS] image b
            off = X0OFF + b * IMB if b < 2 else X2OFF + (b - 2) * IMB
            return inp_sb[:, off:off + IMB].rearrange("p (r s) -> p r s", r=2)

        # HAM pre-warm: run junk matmuls on zeroed SBUF during the input-DMA
        # window so the PE clock gate is already at 2.4GHz (warm) when the
        # first real matmul issues (~3.4us of sustained activity required).
        wj = const.tile([128, 256], f8, tag="wj")
        nc.vector.memset(wj[:], 0.0)
        jl = wj[:].rearrange("p (r m) -> p r m", r=2)
        jp = psum.tile([128, NT], f32, tag="pt", name="jp")
        for _ in range(38):
            nc.tensor.matmul(jp[:, 0:128], jl, jl, start=True, stop=True,
                             perf_mode=DR)

        from concourse.tile_rust import add_dep_helper
        dma_a = nc.sync.dma_start(inp_sb[:, 0:SA1], inpa_d)
        dma_a2 = nc.sync.dma_start(inp_sb[:, SA1:SA], inpa2_d)
        dma_b1 = nc.sync.dma_start(inp_sb[:, SA:SB1], inpb1_d)
        dma_b2 = nc.sync.dma_start(inp_sb[:, SB1:INPB], inpb2_d)
        dma_r = nc.sync.dma_start(rs_sb[:], r_d)
        for a, b in ((dma_b1, dma_a2), (dma_b2, dma_b1), (dma_r, dma_b2)):
            add_dep_helper(a.ins, b.ins, sync=True,
                           reason="stage input DMAs by first-use order")
        nc.gpsimd.memset(x2_sb[:], 0.0)

        def vcol(i):
            return vecv[:, i : i + 1]

        def valid(apnt):   # [128,406] -> [128,14,28] dropping 1 garbage col/row
            return apnt.rearrange("p (h w) -> p h w", w=29)[:, :, :28]

        for ci, src in ((0, None), (1, x2_sb)):
            # conv2 tapers to 1-image phases so the final epilogue tail is short
            groups = ([(b0, 2) for b0 in range(0, BC, 2)] if ci == 0 else
                      [(0, 2), (2, 2), (4, 2), (6, 1), (7, 1)])
            for b0, gsz in groups:
                for cot in range(2):
                    pts = {}
                    # -- 9 taps x (gsz images x 2 row-halves) per weight --
                    for k in range(9):
                        off = (k // 3) * 29 + (k % 3)
                        lhsT = w_ap((ci * 2 + cot) * 9 + k)
                        for bb in range(gsz):
                            b = b0 + bb
                            for hb in range(2):
                                if k == 0:
                                    pts[(bb, hb)] = psum.tile(
                                        [128, NT], f32, tag="pt", name="pt")
                                s = hb * NT + off
                                rhs = (x1_ap(b) if ci == 0 else
                                       src[:, b, :, :])[:, :, s : s + NT]
                                nc.tensor.matmul(
                                    pts[(bb, hb)][:], lhsT, rhs,
                                    start=(k == 0), stop=(k == 8), perf_mode=DR)
                    # ---- epilogues for this phase's psum tiles ----
                    for bb in range(gsz):
                        b = b0 + bb
                        if ci == 1:
                            yb = yp.tile([128, 2, 14, 28], f32, tag="yb", name="yb")
                        for hb in range(2):
                            pt3 = valid(pts[(bb, hb)][:])
                            if ci == 0:
                                # t=P*a1+b1p ; clip +-7 ; +-MAGIC rne -> fp8
                                t1 = ep1.tile([128, 14, 28], f32, tag="t1", name="t1")
                                nc.scalar.activation(
                                    t1[:], pt3, AF.Identity,
                                    bias=vcol(2 + cot), scale=vcol(0 + cot))
                                t2 = ep1.tile([128, 14, 28], f32, tag="t2", name="t2")
                                nc.vector.tensor_scalar(
                                    t2[:], t1[:], 7.0, -7.0, op0=OP.min, op1=OP.max)
                                if bb == 0 and hb == 0 and b0 in (0, 2):
                                    # any tile hitting exactly 7.0 proves
                                    # alpha2 == 1.0 globally (clip bound)
                                    idx = (b0 // 2) * 2 + cot
                                    nc.vector.tensor_reduce(
                                        am_sb[:, idx : idx + 1], t2[:], op=OP.max,
                                        axis=AX.XY, apply_absolute_value=True)
                                t3 = ep1.tile([128, 14, 28], f32, tag="t3", name="t3")
                                nc.scalar.activation(
                                    t3[:], t2[:], AF.Copy, bias=float(MAGIC), scale=1.0)
                                dst = valid(
                                    x2_sb[:, b, cot, hb * NT + 30 : hb * NT + 30 + NT])
                                nc.vector.tensor_scalar(
                                    dst, t3[:], -float(MAGIC), None, op0=OP.add)
                            else:
                                # y = clip(P2*c2 + (x*inv2 + b2), +-1);
                                # the residual affine is precomputed on host
                                u3 = ep2.tile([128, 14, 28], f32, tag="u3", name="u3")
                                nc.vector.scalar_tensor_tensor(
                                    u3[:], pt3, vcol(4 + cot), rs_sb[:, cot, b, hb],
                                    op0=OP.mult, op1=OP.add)
                                nc.vector.tensor_scalar(
                                    yb[:, hb], u3[:], 1.0, -1.0,
                                    op0=OP.min, op1=OP.max)
                                if gsz == 1:
                                    nc.sync.dma_start(y_d[cot, :, b, hb],
                                                      yb[:, hb])
                        if ci == 1 and gsz > 1:
                            nc.sync.dma_start(y_d[cot, :, b], yb[:])
            if ci == 0:
                nc.sync.dma_start(am_d, am_sb[:])

    nc.compile()
    _dedupe_ldweights(nc)
    _BUILT = (nc,)
    return _BUILT


# ------------------------------------------------------------- input packing
def _prep(x, w1, w2, inv1, b1, inv2, b2):
    xi, s_x = _quant_int(x)
    w1i, s_w1 = _quant_int(w1)
    w2i, s_w2 = _quant_int(w2)

    xi8 = xi.astype(F8NP)
    tmp = np.zeros((NCORES, BC, 2, 128, 30, 29), F8NP)
    tmp[:, :, :, :, 1:29, 1:29] = xi8.reshape(NCORES, BC, 2, 128, 28, 28)
    x1_all = np.zeros((NCORES, 128, BC, 2, IMS), F8NP)
    x1_all[..., :870] = tmp.transpose(0, 3, 1, 2, 4, 5).reshape(
        NCORES, 128, BC, 2, 870)

    def wpack(wi):
        # w[cot*128+m, r*128+p, kh, kw] -> [p, (cot,k), r, m]
        v = wi.reshape(2, 128, 2, 128, 9)          # cot, m, r, p, k
        v = v.transpose(3, 0, 4, 2, 1)             # p, cot, k, r, m
        return v.reshape(128, 18, 2, 128).astype(F8NP)

    w_all = np.concatenate([wpack(w1i), wpack(w2i)], axis=1).reshape(128, WB)

    s2 = np.float32(np.float32(1.0) / np.float32(7.0))
    a1 = (np.float32(7.0) * s_x * s_w1 * inv1).astype(np.float32)
    b1p = (np.float32(7.0) * b1).astype(np.float32)
    c2 = (s2 * s_w2 * inv2).astype(np.float32)
    cols = [a1[:128], a1[128:], b1p[:128], b1p[128:], c2[:128], c2[128:],
            inv2[:128], inv2[128:], b2[:128], b2[128:]]
    vec8 = np.zeros((128, VB), F8NP)
    vec8[:, :40] = np.ascontiguousarray(
        np.stack(cols, axis=1).astype(np.float32)).view(F8NP)

    # residual affine x*inv2 + b2, precomputed -> [cores, 128(m), 2(cot), BC, ...]
    rs2 = (x * inv2[None, :, None, None] + b2[None, :, None, None]).astype(np.float32)
    resid = rs2.reshape(NCORES, BC, 2, 128, 2, 14, 28).transpose(0, 3, 2, 1, 4, 5, 6)
    resid = np.ascontiguousarray(resid)

    in_maps = []
    for i in range(NCORES):
        x1i = x1_all[i].reshape(128, X1B)
        inpa = np.concatenate([w_all[:, :W0B], vec8, x1i[:, :IMB]], axis=1)
        inpa2 = x1i[:, IMB:2 * IMB]
        inpb1 = np.concatenate([w_all[:, W0B:], x1i[:, 2 * IMB:4 * IMB]], axis=1)
        inpb2 = x1i[:, 4 * IMB:]
        in_maps.append({"inpa": np.ascontiguousarray(inpa),
                        "inpa2": np.ascontiguousarray(inpa2),
                        "inpb1": np.ascontiguousarray(inpb1),
                        "inpb2": np.ascontiguousarray(inpb2),
                        "resid": resid[i]})
    return in_maps, (xi, w1i, w2i, s_x, s_w1, s_w2, s2)


# ------------------------------------------------------- exact numpy fallback
def _conv3x3_int(xint, wint):
    Bn, Cn, Hn, Wn = xint.shape
    xp = np.zeros((Bn, Cn, Hn + 2, Wn + 2), np.float64)
    xp[:, :, 1:-1, 1:-1] = xint
    out = np.zeros((Bn, wint.shape[0], Hn, Wn), np.float64)
    w64 = wint.astype(np.float64)
    for kh in range(3):
        for kw in range(3):
            out += np.einsum("bchw,oc->bohw", xp[:, :, kh:kh + Hn, kw:kw + Wn],
                             w64[:, :, kh, kw], optimize=True)
    return out.astype(np.float32)


def _numpy_path(x, q, inv1, b1, inv2, b2):
    """Exact replica handling arbitrary alpha2 (never expected to run)."""
    xi, w1i, w2i, s_x, s_w1, s_w2, _ = q
    P1 = _conv3x3_int(xi, w1i)
    h = (P1 * (s_x * s_w1 * inv1)[None, :, None, None]).astype(np.float32)
    h = (h + b1[None, :, None, None]).astype(np.float32)
    h = np.clip(h, np.float32(-1.0), np.float32(1.0))
    alpha2 = np.float32(np.abs(h).max())
    s2 = np.float32(alpha2 / np.float32(7.0))
    x2 = np.round(np.clip(h, -alpha2, alpha2) / s2).astype(np.float32)
    P2 = _conv3x3_int(x2, w2i)
    u = (P2 * (s2 * s_w2 * inv2)[None, :, None, None]).astype(np.float32)
    u = (u + (x * inv2[None, :, None, None] + b2[None, :, None, None])).astype(np.float32)
    return np.clip(u, np.float32(-1.0), np.float32(1.0))


# ------------------------------------------------------------------- kernel
def _dedupe_ldweights(nc):
    """Drop InstLdweights that reload the stationary operand already in the
    PE array (consecutive matmuls here reuse one weight 8x).  Safe because
    Ldweights carry no semaphore updates; ones carrying waits are kept."""
    for f in nc.m.functions:
        for blk in f.blocks:
            il = blk.instructions
            keep, last_sig, removed = [], None, 0
            for ins in il:
                tn = type(ins).__name__
                if tn == "InstLdweights":
                    sig = (str(ins.ins), str(ins.perf_mode),
                           str(ins.tile_position), str(ins.is_transpose))
                    plain = ("wait:" not in str(ins)
                             and "update:" not in str(ins))
                    if sig == last_sig and plain:
                        removed += 1
                        continue
                    last_sig = sig
                elif tn in ("InstMatmult", "InstEventSemaphore", "InstDrain"):
                    pass                     # none of these clobber loaded weights
                elif str(getattr(ins, "engine", "")).endswith("PE"):
                    last_sig = None          # conservative reset on other PE ops
                keep.append(ins)
            if removed:
                il[:] = keep


def _run(in_maps, trace=False, tmpdir=None):
    from concourse.bass_utils import run_bass_kernel_spmd
    (nc,) = _build()
    return run_bass_kernel_spmd(nc, in_maps, list(range(NCORES)), trace=trace,
                                tmpdir=tmpdir)


def kernel(x, w1, bn1_gamma, bn1_beta, bn1_mean, bn1_var,
           w2, bn2_gamma, bn2_beta, bn2_mean, bn2_var):
    x = np.asarray(x, np.float32)
    w1 = np.asarray(w1, np.float32)
    w2 = np.asarray(w2, np.float32)
    inv1, b1 = _fold_bn(bn1_gamma, bn1_beta, bn1_mean, bn1_var)
    inv2, b2 = _fold_bn(bn2_gamma, bn2_beta, bn2_mean, bn2_var)

    in_maps, q = _prep(x, w1, w2, inv1, b1, inv2, b2)
    res = _run(in_maps)

    amax = np.max([r["amax"] for r in res.results])
    if not np.float32(amax) == np.float32(7.0):
        return _numpy_path(x, q, inv1, b1, inv2, b2)

    ys = np.stack([r["y"] for r in res.results])      # [cores, 2, 128, BC, 2,14,28]
    ys = ys.reshape(NCORES, 2, 128, BC, 784)
    return ys.transpose(0, 3, 1, 2, 4).reshape(B, C, H, W).copy()


# revision 28
# speedup vs baseline: 1.0519x; 1.0046x over previous
"""Trainium2 Bass kernel for the quantized BasicBlock (nn_BasicBlock_15436112462307).

Strategy
--------
Data-parallel over batch: 64 images -> 8 cores x 8 images. Weights/BN replicated.

fake_quant makes every conv operand an exact small integer (-7..7) times a
global fp32 scale.  We factor the scales out on the host and feed pure
integers to the PE as fp8e4 (integers <=7 are exact in fp8e4), using
perf_mode=DoubleRow so one matmul contracts all 256 input channels
(lhsT [128,2,128] / rhs [128,2,N]) at 2x fp8 rate.  PSUM accumulates the
integer dot products exactly in fp32, so the conv itself is EXACT; all
rounding happens only in the per-channel epilogues, which replicate the
reference's fp32 arithmetic.

Spatial layout: each 28x28 image is zero-padded to 30 rows x 29 cols and
flattened; ONE zero column is shared as the right-pad of row h and the
left-pad of row h+1, so every 3x3 conv tap is a pure diagonal shift in the
flat index -> conv = 9 accumulating matmuls over contiguous windows, with
only 1 garbage column per 29 discarded in the epilogue APs.

Epilogue 1 (conv1 -> conv2 input):  q2 = rne(clip(P1*(7*sx*sw1*inv1) + 7*b1, +-7))
using the fp32 magic-number trick (+-1.5*2^23) for round-to-nearest-even;
the result is an exact integer written directly as fp8 into the padded conv2
input buffer.  The activation fake-quant scale alpha2 = max|hardtanh(...)| is
1.0 whenever anything clips (always, for this distribution); the kernel
computes max|.| on device and the host verifies it is exactly 7.0, falling
back to an exact numpy implementation otherwise.

Epilogue 2: y = clip(P2*(s2*sw2*inv2) + (x*inv2 + b2), +-1); the residual
affine x*inv2+b2 is precomputed on the host, so the device does one fused
scalar_tensor_tensor (scale+add) plus the clip on VectorE.

Inputs stream in staged DMAs ordered by first use (conv1/cot0 weights +
images 0-1 in two parallel transfers, the rest chained behind).  Junk
matmuls on zeroed SBUF fill the DMA wait so the PE HAM clock-gate is warm
(2.4GHz) when real matmuls start ~11us in.  Measured ~117us: the tensor
engine runs its 576 DoubleRow matmuls at the streaming floor (~99us,
zero stalls, one HAM transition), conv2 tapers to 1-image phases so the
epilogue tail is ~2us, and the rest is fixed preamble + exit barrier.
"""

import numpy as np
import ml_dtypes

EPS = np.float32(1e-5)
NCORES = 8
B, C, H, W = 64, 256, 28, 28
BC = B // NCORES            # images per core
IMS = 880                   # padded (30 rows x 29 cols = 870) image stride;
                            # one zero col shared as right-pad of row h and
                            # left-pad of row h+1
NT = 406                    # matmul N: 14 padded rows x 29
MAGIC = np.float32(12582912.0)  # 1.5 * 2^23
F8NP = ml_dtypes.float8_e4m3

WB = 36 * 2 * 128                   # 9216 bytes/partition of int weights
WG = 2 * 128                        # one weight group (tap)
VB = 48                             # 40B of fp32 epilogue vecs + 8B pad
IMB = 2 * IMS                       # one image (both channel halves)
X1B = BC * IMB                      # 14592 bytes/partition of int inputs
# staged layout: [w(conv1,cot0) | vec | imgs0-1 | w rest | imgs2-7]
W0B = 9 * WG
VOFF = W0B
X0OFF = W0B + VB
WROFF = X0OFF + 2 * IMB
X2OFF = WROFF + (WB - W0B)
INPB = X2OFF + 6 * IMB

_BUILT = None  # cached (nc,) so repeat calls skip IR building + compile


# ----------------------------------------------------------------- host math
def _quant_int(v):
    """Exact replica of the reference fake_quant grid; returns integer part."""
    alpha = np.float32(np.float32(np.max(np.abs(v))) + np.float32(1e-12))
    scale = np.float32(alpha / np.float32(7.0))
    q = np.round(np.clip(v, -alpha, alpha) / scale).astype(np.float32)
    return q, scale


def _fold_bn(gamma, beta, mean, var):
    gamma = np.asarray(gamma, np.float32)
    beta = np.asarray(beta, np.float32)
    mean = np.asarray(mean, np.float32)
    var = np.asarray(var, np.float32)
    inv = (gamma / np.sqrt(var + EPS)).astype(np.float32)
    b = (beta - mean * inv).astype(np.float32)
    return inv, b


# ------------------------------------------------------------------ bass IR
def _build():
    global _BUILT
    if _BUILT is not None:
        return _BUILT
    import concourse.bacc as bacc
    import concourse.tile as tile
    from concourse import mybir
    from contextlib import ExitStack

    f32 = mybir.dt.float32
    f8 = mybir.dt.float8e4
    AF = mybir.ActivationFunctionType
    OP = mybir.AluOpType
    DR = mybir.MatmulPerfMode.DoubleRow
    AX = mybir.AxisListType

    SA1, SA, SB1 = X0OFF + IMB, WROFF, X2OFF + 2 * IMB
    nc = bacc.Bacc("TRN2", target_bir_lowering=False, debug=False)
    inpa_d = nc.dram_tensor("inpa", [128, SA1], f8, kind="ExternalInput").ap()
    inpa2_d = nc.dram_tensor("inpa2", [128, SA - SA1], f8, kind="ExternalInput").ap()
    inpb1_d = nc.dram_tensor("inpb1", [128, SB1 - SA], f8, kind="ExternalInput").ap()
    inpb2_d = nc.dram_tensor("inpb2", [128, INPB - SB1], f8, kind="ExternalInput").ap()
    r_d = nc.dram_tensor("resid", [128, 2, BC, 2, 14, 28], f32, kind="ExternalInput").ap()
    y_d = nc.dram_tensor("y", [2, 128, BC, 2, 14, 28], f32, kind="ExternalOutput").ap()
    am_d = nc.dram_tensor("amax", [128, 4], f32, kind="ExternalOutput").ap()

    with tile.TileContext(nc) as tc, ExitStack() as ctx:
        const = ctx.enter_context(tc.tile_pool(name="const", bufs=1))
        psum = ctx.enter_context(tc.tile_pool(name="psum", bufs=8, space="PSUM"))
        ep1 = ctx.enter_context(tc.tile_pool(name="ep1", bufs=4))
        ep2 = ctx.enter_context(tc.tile_pool(name="ep2", bufs=4))
        yp = ctx.enter_context(tc.tile_pool(name="yp", bufs=3))

        inp_sb = const.tile([128, INPB], f8, tag="inp")
        x2_sb = const.tile([128, BC, 2, IMS], f8, tag="x2")
        rs_sb = const.tile([128, 2, BC, 2, 14, 28], f32, tag="rs")
        am_sb = const.tile([128, 4], f32, tag="am")

        vecv = inp_sb[:, VOFF:VOFF + 40].bitcast(f32)      # [128, 10] f32

        def w_ap(g):   # [128, 2, 128] tap g in (ci,cot)-major order
            off = g * WG if g < 9 else WROFF + (g - 9) * WG
            return inp_sb[:, off:off + WG].rearrange("p (r m) -> p r m", r=2)

        def x1_ap(b):  # [128, 2, IMS] image b
            off = X0OFF + b * IMB if b < 2 else X2OFF + (b - 2) * IMB
            return inp_sb[:, off:off + IMB].rearrange("p (r s) -> p r s", r=2)

        # HAM pre-warm: run junk matmuls on zeroed SBUF during the input-DMA
        # window so the PE clock gate is already at 2.4GHz (warm) when the
        # first real matmul issues (~3.4us of sustained activity required).
        wj = const.tile([128, 256], f8, tag="wj")
        nc.vector.memset(wj[:], 0.0)
        jl = wj[:].rearrange("p (r m) -> p r m", r=2)
        jp = psum.tile([128, NT], f32, tag="pt", name="jp")
        for _ in range(38):
            nc.tensor.matmul(jp[:, 0:128], jl, jl, start=True, stop=True,
                             perf_mode=DR)

        from concourse.tile_rust import add_dep_helper
        dma_a = nc.sync.dma_start(inp_sb[:, 0:SA1], inpa_d)
        dma_a2 = nc.sync.dma_start(inp_sb[:, SA1:SA], inpa2_d)
        dma_b1 = nc.sync.dma_start(inp_sb[:, SA:SB1], inpb1_d)
        dma_b2 = nc.sync.dma_start(inp_sb[:, SB1:INPB], inpb2_d)
        dma_r = nc.sync.dma_start(rs_sb[:], r_d)
        for a, b in ((dma_b1, dma_a2), (dma_b2, dma_b1), (dma_r, dma_b2)):
            add_dep_helper(a.ins, b.ins, sync=True,
                           reason="stage input DMAs by first-use order")
        nc.gpsimd.memset(x2_sb[:], 0.0)

        def vcol(i):
            return vecv[:, i : i + 1]

        def valid(apnt):   # [128,406] -> [128,14,28] dropping 1 garbage col/row
            return apnt.rearrange("p (h w) -> p h w", w=29)[:, :, :28]

        for ci, src in ((0, None), (1, x2_sb)):
            # conv2 tapers to 1-image phases so the final epilogue tail is short
            groups = ([(b0, 2) for b0 in range(0, BC, 2)] if ci == 0 else
                      [(0, 2), (2, 2), (4, 2), (6, 1), (7, 1)])
            for b0, gsz in groups:
                for cot in range(2):
                    pts = {}
                    # -- 9 taps x (gsz images x 2 row-halves) per weight --
                    for k in range(9):
                        off = (k // 3) * 29 + (k % 3)
                        lhsT = w_ap((ci * 2 + cot) * 9 + k)
                        for bb in range(gsz):
                            b = b0 + bb
                            for hb in range(2):
                                if k == 0:
                                    pts[(bb, hb)] = psum.tile(
                                        [128, NT], f32, tag="pt", name="pt")
                                s = hb * NT + off
                                rhs = (x1_ap(b) if ci == 0 else
                                       src[:, b, :, :])[:, :, s : s + NT]
                                nc.tensor.matmul(
                                    pts[(bb, hb)][:], lhsT, rhs,
                                    start=(k == 0), stop=(k == 8), perf_mode=DR)
                    # ---- epilogues for this phase's psum tiles ----
                    for bb in range(gsz):
                        b = b0 + bb
                        if ci == 1:
                            yb = yp.tile([128, 2, 14, 28], f32, tag="yb", name="yb")
                        for hb in range(2):
                            pt3 = valid(pts[(bb, hb)][:])
                            if ci == 0:
                                # t=P*a1+b1p ; clip +-7 ; +-MAGIC rne -> fp8
                                t1 = ep1.tile([128, 14, 28], f32, tag="t1", name="t1")
                                nc.scalar.activation(
                                    t1[:], pt3, AF.Identity,
                                    bias=vcol(2 + cot), scale=vcol(0 + cot))
                                t2 = ep1.tile([128, 14, 28], f32, tag="t2", name="t2")
                                nc.vector.tensor_scalar(
                                    t2[:], t1[:], 7.0, -7.0, op0=OP.min, op1=OP.max)
                                if bb == 0 and hb == 0 and b0 in (0, 2):
                                    # any tile hitting exactly 7.0 proves
                                    # alpha2 == 1.0 globally (clip bound)
                                    idx = (b0 // 2) * 2 + cot
                                    nc.vector.tensor_reduce(
                                        am_sb[:, idx : idx + 1], t2[:], op=OP.max,
                                        axis=AX.XY, apply_absolute_value=True)
                                t3 = ep1.tile([128, 14, 28], f32, tag="t3", name="t3")
                                nc.scalar.activation(
                                    t3[:], t2[:], AF.Copy, bias=float(MAGIC), scale=1.0)
                                dst = valid(
                                    x2_sb[:, b, cot, hb * NT + 30 : hb * NT + 30 + NT])
                                nc.vector.tensor_scalar(
                                    dst, t3[:], -float(MAGIC), None, op0=OP.add)
                            else:
                                # y = clip(P2*c2 + (x*inv2 + b2), +-1);
                                # the residual affine is precomputed on host
                                u3 = ep2.tile([128, 14, 28], f32, tag="u3", name="u3")
                                nc.vector.scalar_tensor_tensor(
                                    u3[:], pt3, vcol(4 + cot), rs_sb[:, cot, b, hb],
                                    op0=OP.mult, op1=OP.add)
                                nc.vector.tensor_scalar(
                                    yb[:, hb], u3[:], 1.0, -1.0,
                                    op0=OP.min, op1=OP.max)
                                if gsz == 1:
                                    nc.sync.dma_start(y_d[cot, :, b, hb],
                                                      yb[:, hb])
                        if ci == 1 and gsz > 1:
                            nc.sync.dma_start(y_d[cot, :, b], yb[:])
            if ci == 0:
                nc.sync.dma_start(am_d, am_sb[:])

    nc.compile()
    _dedupe_ldweights(nc)
    _BUILT = (nc,)
    return _BUILT


# ------------------------------------------------------------- input packing
def _prep(x, w1, w2, inv1, b1, inv2, b2):
    xi, s_x = _quant_int(x)
    w1i, s_w1 = _quant_int(w1)
    w2i, s_w2 = _quant_int(w2)

    xi8 = xi.astype(F8NP)
    tmp = np.zeros((NCORES, BC, 2, 128, 30, 29), F8NP)
    tmp[:, :, :, :, 1:29, 1:29] = xi8.reshape(NCORES, BC, 2, 128, 28, 28)
    x1_all = np.zeros((NCORES, 128, BC, 2, IMS), F8NP)
    x1_all[..., :870] = tmp.transpose(0, 3, 1, 2, 4, 5).reshape(
        NCORES, 128, BC, 2, 870)

    def wpack(wi):
        # w[cot*128+m, r*128+p, kh, kw] -> [p, (cot,k), r, m]
        v = wi.reshape(2, 128, 2, 128, 9)          # cot, m, r, p, k
        v = v.transpose(3, 0, 4, 2, 1)             # p, cot, k, r, m
        return v.reshape(128, 18, 2, 128).astype(F8NP)

    w_all = np.concatenate([wpack(w1i), wpack(w2i)], axis=1).reshape(128, WB)

    s2 = np.float32(np.float32(1.0) / np.float32(7.0))
    a1 = (np.float32(7.0) * s_x * s_w1 * inv1).astype(np.float32)
    b1p = (np.float32(7.0) * b1).astype(np.float32)
    c2 = (s2 * s_w2 * inv2).astype(np.float32)
    cols = [a1[:128], a1[128:], b1p[:128], b1p[128:], c2[:128], c2[128:],
            inv2[:128], inv2[128:], b2[:128], b2[128:]]
    vec8 = np.zeros((128, VB), F8NP)
    vec8[:, :40] = np.ascontiguousarray(
        np.stack(cols, axis=1).astype(np.float32)).view(F8NP)

    # residual affine x*inv2 + b2, precomputed -> [cores, 128(m), 2(cot), BC, ...]
    rs2 = (x * inv2[None, :, None, None] + b2[None, :, None, None]).astype(np.float32)
    resid = rs2.reshape(NCORES, BC, 2, 128, 2, 14, 28).transpose(0, 3, 2, 1, 4, 5, 6)
    resid = np.ascontiguousarray(resid)

    in_maps = []
    for i in range(NCORES):
        x1i = x1_all[i].reshape(128, X1B)
        inpa = np.concatenate([w_all[:, :W0B], vec8, x1i[:, :IMB]], axis=1)
        inpa2 = x1i[:, IMB:2 * IMB]
        inpb1 = np.concatenate([w_all[:, W0B:], x1i[:, 2 * IMB:4 * IMB]], axis=1)
        inpb2 = x1i[:, 4 * IMB:]
        in_maps.append({"inpa": np.ascontiguousarray(inpa),
                        "inpa2": np.ascontiguousarray(inpa2),
                        "inpb1": np.ascontiguousarray(inpb1),
                        "inpb2": np.ascontiguousarray(inpb2),
                        "resid": resid[i]})
    return in_maps, (xi, w1i, w2i, s_x, s_w1, s_w2, s2)


# ------------------------------------------------------- exact numpy fallback
def _conv3x3_int(xint, wint):
    Bn, Cn, Hn, Wn = xint.shape
    xp = np.zeros((Bn, Cn, Hn + 2, Wn + 2), np.float64)
    xp[:, :, 1:-1, 1:-1] = xint
    out = np.zeros((Bn, wint.shape[0], Hn, Wn), np.float64)
    w64 = wint.astype(np.float64)
    for kh in range(3):
        for kw in range(3):
            out += np.einsum("bchw,oc->bohw", xp[:, :, kh:kh + Hn, kw:kw + Wn],
                             w64[:, :, kh, kw], optimize=True)
    return out.astype(np.float32)


def _numpy_path(x, q, inv1, b1, inv2, b2):
    """Exact replica handling arbitrary alpha2 (never expected to run)."""
    xi, w1i, w2i, s_x, s_w1, s_w2, _ = q
    P1 = _conv3x3_int(xi, w1i)
    h = (P1 * (s_x * s_w1 * inv1)[None, :, None, None]).astype(np.float32)
    h = (h + b1[None, :, None, None]).astype(np.float32)
    h = np.clip(h, np.float32(-1.0), np.float32(1.0))
    alpha2 = np.float32(np.abs(h).max())
    s2 = np.float32(alpha2 / np.float32(7.0))
    x2 = np.round(np.clip(h, -alpha2, alpha2) / s2).astype(np.float32)
    P2 = _conv3x3_int(x2, w2i)
    u = (P2 * (s2 * s_w2 * inv2)[None, :, None, None]).astype(np.float32)
    u = (u + (x * inv2[None, :, None, None] + b2[None, :, None, None])).astype(np.float32)
    return np.clip(u, np.float32(-1.0), np.float32(1.0))


# ------------------------------------------------------------------- kernel
def _dedupe_ldweights(nc):
    """Drop InstLdweights that reload the stationary operand already in the
    PE array (consecutive matmuls here reuse one weight 8x).  Safe because
    Ldweights carry no semaphore updates; ones carrying waits are kept."""
    for f in nc.m.functions:
        for blk in f.blocks:
            il = blk.instructions
            keep, last_sig, removed = [], None, 0
            for ins in il:
                tn = type(ins).__name__
                if tn == "InstLdweights":
                    sig = (str(ins.ins), str(ins.perf_mode),
                           str(ins.tile_position), str(ins.is_transpose))
                    plain = ("wait:" not in str(ins)
                             and "update:" not in str(ins))
                    if sig == last_sig and plain:
                        removed += 1
                        continue
                    last_sig = sig
                elif tn in ("InstMatmult", "InstEventSemaphore", "InstDrain"):
                    pass                     # none of these clobber loaded weights
                elif str(getattr(ins, "engine", "")).endswith("PE"):
                    last_sig = None          # conservative reset on other PE ops
                keep.append(ins)
            if removed:
                il[:] = keep


def _run(in_maps, trace=False, tmpdir=None):
    from concourse.bass_utils import run_bass_kernel_spmd
    (nc,) = _build()
    return run_bass_kernel_spmd(nc, in_maps, list(range(NCORES)), trace=trace,
                                tmpdir=tmpdir)


def kernel(x, w1, bn1_gamma, bn1_beta, bn1_mean, bn1_var,
           w2, bn2_gamma, bn2_beta, bn2_mean, bn2_var):
    x = np.asarray(x, np.float32)
    w1 = np.asarray(w1, np.float32)
    w2 = np.asarray(w2, np.float32)
    inv1, b1 = _fold_bn(bn1_gamma, bn1_beta, bn1_mean, bn1_var)
    inv2, b2 = _fold_bn(bn2_gamma, bn2_beta, bn2_mean, bn2_var)

    in_maps, q = _prep(x, w1, w2, inv1, b1, inv2, b2)
    res = _run(in_maps)

    amax = np.max([r["amax"] for r in res.results])
    if not np.float32(amax) == np.float32(7.0):
        return _numpy_path(x, q, inv1, b1, inv2, b2)

    ys = np.stack([r["y"] for r in res.results])      # [cores, 2, 128, BC, 2,14,28]
    ys = ys.reshape(NCORES, 2, 128, BC, 784)
    return ys.transpose(0, 3, 1, 2, 4).reshape(B, C, H, W).copy()
